# revision 6
# baseline (speedup 1.0000x reference)
"""Trainium2 Bass kernel for nn_Block_50130858279051 (dense transformer block).

Sharding: 8 cores = 2 batch groups x 4-way tensor parallel (as v1), with a
restructured schedule:
  - Attention runs PAIR-MAJOR: all 4 query slices for head-pair 0, fire its
    8-rank AllToAll, then pair 1 while the collective flies, then A2A #1.
  - Scores for the two heads of a pair are issued back-to-back into row
    groups 0/64 of the PE array (tile_position auto-derived), so they run
    concurrently; one [P,2,512] Exp covers both heads.
  - Diagonal k-tiles only process query columns >= 128*j (the causal-valid
    range), shrinking scores/exp/mask/AV on the diagonal by ~37%.
  - proj reuses each ylt lhsT across both 512-column halves (one LDWEIGHTS
    per two matmuls) and is split pair-0-first so its first half runs while
    A2A #1 is still in flight.
  - fc2 runs s-major with each hT stationary tile reused for both output
    halves, accumulating into 8 PSUM banks opened in their own pool scope.
  - All xb token blocks prefetch at kernel start (block 0 first).
"""
import sys

sys.path.insert(0, "/opt/trn_rl_repo")

import numpy as np
import ml_dtypes
from contextlib import ExitStack

import concourse.bacc as bacc
import concourse.mybir as mybir
import concourse.tile as tile
from concourse.bass_utils import run_bass_kernel_spmd

B, T, C, H, HD = 2, 2048, 1024, 16, 64
HID = 4 * C
P = 128
NCORES, TPG = 8, 4          # 2 groups x 4 cores
TCHUNK = T // TPG           # 512 tokens per core in the MLP phase
HPC = H // TPG              # 4 heads per core
CS = C // P                 # 8 channel subtiles
TT = T // P                 # 16 token tiles
NQ = T // 512               # 4 query slices of 512
NT2 = TCHUNK // P           # 4 token tiles in the chunk
NS = HID // P               # 32 hidden subtiles
f32, bf16 = mybir.dt.float32, mybir.dt.bfloat16
BF = ml_dtypes.bfloat16
ACT = mybir.ActivationFunctionType


def build_nc(debug=False, sim_mode=False, do_p1=True, do_p2=True):
    nc = bacc.Bacc("TRN2", target_bir_lowering=False, debug=False,
                   num_devices=NCORES, num_swdge_queues=4)
    xb = nc.declare_dram_parameter("xb", [T, C], bf16, isOutput=False)
    xc = nc.declare_dram_parameter("xc", [TCHUNK, C], bf16, isOutput=False)
    wq = nc.declare_dram_parameter("wq", [P, CS, 2 * P], bf16, isOutput=False)
    wk = nc.declare_dram_parameter("wk", [P, CS, 2 * P], bf16, isOutput=False)
    wv = nc.declare_dram_parameter("wv", [P, CS, 2 * P], bf16, isOutput=False)
    pw = nc.declare_dram_parameter("pw", [P, 2 * CS, C], bf16, isOutput=False)
    fw = nc.declare_dram_parameter("fw", [16, P, CS, 256], bf16,
                                   isOutput=False)
    f2w = nc.declare_dram_parameter("f2w", [16, P, 2, C], bf16,
                                    isOutput=False)
    consts = nc.declare_dram_parameter("consts", [P, P + 4 * 512], bf16,
                                       isOutput=False)
    out = nc.declare_dram_parameter("out", [TCHUNK, C], bf16, isOutput=True)

    with tile.TileContext(nc) as tc, ExitStack() as ctx:
        per = ctx.enter_context(tc.tile_pool(name="persist", bufs=1))
        work = ctx.enter_context(tc.tile_pool(name="work", bufs=3))
        small = ctx.enter_context(tc.tile_pool(name="small", bufs=3))
        dram = ctx.enter_context(tc.tile_pool(name="dram", bufs=1,
                                              space="DRAM"))

        # ---- constants: ident + causal diagonal masks --------------------
        # ident rides first (the first LN tile needs it within ~3us);
        # masks aren't needed until attention starts.
        cst = per.tile([P, P + 4 * 512], bf16, tag="cst")
        nc.gpsimd.dma_start(cst[:, 0:P], consts[:, 0:P])
        ident = cst[:, 0:P]
        masks = cst[:, P:].rearrange("p (j t) -> p j t", j=4)
        eps_t = per.tile([P, 1], f32, tag="eps")
        nc.vector.memset(eps_t[:], 1e-5)

        fws = ctx.enter_context(tc.tile_pool(name="fwstream", bufs=2))
        f2s = ctx.enter_context(tc.tile_pool(name="f2stream", bufs=2))
        ypool = ctx.enter_context(tc.tile_pool(name="ypool", bufs=1))
        engs = [nc.gpsimd, nc.gpsimd, nc.sync, nc.sync]
        fw_chunks = [fws.tile([P, CS, 256], bf16, tag="fwch", name=f"fwch{g}")
                     for g in range(16)]
        f2_chunks = [f2s.tile([P, 2, C], bf16, tag="f2ch", name=f"f2ch{i}")
                     for i in range(16)]

        ylt = [ypool.tile([P, CS, TCHUNK], bf16, tag=f"ylt{pr}",
                          name=f"ylt{pr}") for pr in range(2)]
        ct = ypool.tile([P, NT2, C], bf16, tag="xct")
        pw_s = ypool.tile([P, 2 * CS, C], bf16, tag="pw")

        def issue_prefetch():
            for g in range(8):
                engs[g % 4].dma_start(fw_chunks[g][:], fw[g])
            for idx in range(2):
                engs[idx % 4].dma_start(f2_chunks[idx][:], f2w[idx])
            nc.sync.dma_start(
                ct[:], xc[:].rearrange("(i p) c -> p i c", p=P))
            pwr_d = pw[:].rearrange("p (j two) c -> p two j c", two=2)
            pwr_s = pw_s[:].rearrange("p (j two) c -> p two j c", two=2)
            nc.gpsimd.dma_start(pwr_s[:, 0], pwr_d[:, 0])
            nc.sync.dma_start(pwr_s[:, 1], pwr_d[:, 1])

        ybounce = [dram.tile([NCORES * P, TCHUNK], bf16, name=f"ybounce{i}",
                             tag=f"ybounce{i}") for i in range(2)]
        a2a_out = [dram.tile([NCORES * P, TCHUNK], bf16, name=f"a2a_out{i}",
                             tag=f"a2a_out{i}") for i in range(2)]

        def layernorm_transpose(get_tile, n_tiles, dst, i0, pp):
            """Token-major [P, C] tiles -> feature-major bf16 dst."""
            for i in range(i0, i0 + n_tiles):
                xt = get_tile(i)
                stats = small.tile([P, 2, 6], f32, tag="s_bn")
                mv = small.tile([P, 2], f32, tag="s_mv")
                std = small.tile([P, 1], f32, tag="s_std")
                rstd = small.tile([P, 1], f32, tag="s_rstd")
                negmu = small.tile([P, 1], f32, tag="s_negmu")
                cen = work.tile([P, C], bf16, tag="cen")
                for g in range(2):
                    nc.vector.bn_stats(stats[:, g, :],
                                       xt[:, g * 512:(g + 1) * 512])
                nc.vector.bn_aggr(mv[:],
                                  stats[:].rearrange("p g s -> p (g s)"))
                nc.scalar.activation(std[:], mv[:, 1:2], ACT.Sqrt,
                                     bias=eps_t[:])
                nc.vector.reciprocal(rstd[:], std[:])
                nc.vector.tensor_scalar_mul(negmu[:], mv[:, 0:1], -1.0)
                # centered copy alternates ACT/DVE to balance engine load
                if i % 2 == 0:
                    nc.scalar.activation(cen[:], xt[:], ACT.Identity,
                                         bias=negmu[:])
                else:
                    nc.vector.tensor_scalar_add(cen[:], xt[:], negmu[:])
                dmat = work.tile([P, P], bf16, tag="dmat")
                nc.vector.tensor_scalar_mul(dmat[:], ident, rstd[:])
                for half in range(2):
                    ps = pp.tile([P, 512], f32, tag="mm")
                    for cq in range(4):
                        c = half * 4 + cq
                        nc.tensor.matmul(
                            ps[:, cq * P:(cq + 1) * P],
                            lhsT=cen[:, c * P:(c + 1) * P],
                            rhs=dmat[:], start=True, stop=True)
                    eng = nc.vector if half == 0 else nc.scalar
                    if eng is nc.vector:
                        eng.tensor_copy(
                            out=dst[:, half * 4:(half + 1) * 4,
                                    i * P:(i + 1) * P],
                            in_=ps[:].rearrange("p (c t) -> p c t", c=4))
                    else:
                        eng.copy(
                            dst[:, half * 4:(half + 1) * 4,
                                i * P:(i + 1) * P],
                            ps[:].rearrange("p (c t) -> p c t", c=4))

        # ================= phase 1: LN1, QKV, attention =================
        with tc.tile_pool(name="xlt_pool", bufs=1) as xlt_pool, \
             tc.tile_pool(name="at_pool", bufs=3) as at_pool, \
             tc.tile_pool(name="xb_pool", bufs=3) as xb_pool, \
             tc.tile_pool(name="psy", bufs=2, space="PSUM") as psy, \
             tc.tile_pool(name="psatt", bufs=2, space="PSUM") as psatt, \
             tc.tile_pool(name="psmm", bufs=2, space="PSUM") as psmm:
            xlt = xlt_pool.tile([P, CS, T], bf16, tag="xlt")
            vA = xlt_pool.tile([P, TT, HPC, 65], bf16, tag="vA")
            nc.vector.memset(vA[:, :, :, 64:65], 1.0)
            qT = [xlt_pool.tile([P, T], bf16, tag=f"qT{p}", name=f"qT{p}")
                  for p in range(2)]
            kT = [xlt_pool.tile([P, T], bf16, tag=f"kT{p}", name=f"kT{p}")
                  for p in range(2)]
            ysb = [xlt_pool.tile([P, T], bf16, tag=f"ysb{p}",
                                 name=f"ysb{p}") for p in range(2)]
            wq_s = xlt_pool.tile([P, CS, 2 * P], bf16, tag="wq")
            wk_s = xlt_pool.tile([P, CS, 2 * P], bf16, tag="wk")
            wv_s = xlt_pool.tile([P, CS, 2 * P], bf16, tag="wv")

            # startup DMA: first xb block, qkv weights, then the rest of xb.
            # blocks rotate through 3 buffers; block 3's DMA waits for
            # block 0's consumers automatically.
            # qkv weights ride the gpsimd queue (behind consts only, wv
            # first since v-tiles consume it first); all xb blocks ride
            # the sync queue so they never delay the weights.
            xb_blocks = [xb_pool.tile([P, 4, C], bf16, tag="xbblk",
                                      name=f"xbblk{b}") for b in range(4)]
            xbr = xb[:].rearrange("(blk i p) c -> blk p i c", blk=4, p=P)
            nc.sync.dma_start(xb_blocks[0][:, 0:1, :], xbr[0][:, 0:1, :])
            # PE warm-up: dependency-free matmuls on the identity block fill
            # the otherwise-idle window while the first LN chain's DVE/ACT
            # latency drains, and push the PE clock gate to full rate before
            # the first real matmuls issue.
            wups = psmm.tile([64, P], f32, tag="mm", name="warmup_ps")
            for wi in range(24):
                nc.tensor.matmul(wups[:], lhsT=ident[0:64, 0:64],
                                 rhs=ident[0:64, :],
                                 start=(wi == 0), stop=(wi == 23))
            nc.gpsimd.dma_start(wv_s[:], wv[:])
            nc.gpsimd.dma_start(wq_s[:], wq[:])
            nc.gpsimd.dma_start(wk_s[:], wk[:])
            nc.gpsimd.dma_start(cst[:, P:], consts[:, P:])
            nc.sync.dma_start(xb_blocks[0][:, 1:4, :], xbr[0][:, 1:4, :])
            for blk in range(1, 4):
                nc.sync.dma_start(xb_blocks[blk][:], xbr[blk])

            def xb_tile(i):
                return xb_blocks[i // 4][:, i % 4, :]

            def qk_slice(ts, pair):
                for dst_t, wsb in ((qT[pair], wq_s), (kT[pair], wk_s)):
                    ps = psmm.tile([P, 512], f32, tag="mm")
                    for s in range(CS):
                        nc.tensor.matmul(
                            ps[:],
                            lhsT=wsb[:, s, pair * P:(pair + 1) * P],
                            rhs=xlt[:, s, ts * 512:(ts + 1) * 512],
                            start=(s == 0), stop=(s == CS - 1))
                    nc.vector.tensor_copy(
                        out=dst_t[:, ts * 512:(ts + 1) * 512],
                        in_=ps[:])

            def v_tile(ti):
                ps = psmm.tile([P, 512], f32, tag="mm")
                for s in range(CS):
                    nc.tensor.matmul(
                        ps[:, :2 * P],
                        lhsT=xlt[:, s, ti * P:(ti + 1) * P],
                        rhs=wv_s[:, s, :],
                        start=(s == 0), stop=(s == CS - 1))
                nc.vector.tensor_copy(
                    out=vA[:, ti, :, 0:64],
                    in_=ps[:, :2 * P].rearrange("p (h d) -> p h d",
                                                h=HPC))

            ybr2 = [yb[:].rearrange("(j p) t -> j p t", j=NCORES)
                    for yb in ybounce]

            def attn_unit(qs, pair):
                """Scores (row-packed pair), exp, AV (2 k-tiles behind so
                the packed scores + prior AV cover the exp latency),
                normalize, ship."""
                nkt = 4 * qs + 4
                yps = [psy.tile([65, 512], f32, tag="yps",
                                name=f"yps_{qs}_{pair}_{hp}")
                       for hp in range(2)]

                def av(at, kt, qlo):
                    for hp in range(2):
                        nc.tensor.matmul(
                            yps[hp][:, qlo:],
                            lhsT=vA[:, kt, 2 * pair + hp, :],
                            rhs=at[:, hp, qlo:],
                            start=(kt == 0), stop=(kt == nkt - 1))

                pipe = []
                for kt in range(nkt):
                    j = kt - 4 * qs
                    qlo = max(0, 128 * j)
                    sps = psatt.tile([P, 2, 512], f32, tag="satt")
                    for hp in range(2):
                        nc.tensor.matmul(
                            sps[:, hp, qlo:],
                            lhsT=kT[pair][hp * 64:(hp + 1) * 64,
                                          kt * P:(kt + 1) * P],
                            rhs=qT[pair][hp * 64:(hp + 1) * 64,
                                         qs * 512 + qlo:(qs + 1) * 512],
                            start=True, stop=True)
                    if len(pipe) >= 2:
                        av(*pipe.pop(0))
                    at = at_pool.tile([P, 2, 512], bf16, tag="at")
                    nc.scalar.activation(at[:, :, qlo:], sps[:, :, qlo:],
                                         ACT.Exp)
                    if j >= 0:
                        for hp in range(2):
                            nc.vector.tensor_mul(at[:, hp, qlo:],
                                                 at[:, hp, qlo:],
                                                 masks[:, j, qlo:])
                    pipe.append((at, kt, qlo))
                for p in pipe:
                    av(*p)
                for hp in range(2):
                    lrec = small.tile([1, 512], f32, tag="lrec")
                    nc.vector.reciprocal(lrec[:], yps[hp][64:65, :])
                    rsb = work.tile([64, 512], f32, tag="rsb")
                    nc.gpsimd.partition_broadcast(rsb[:], lrec[:])
                    nc.vector.tensor_mul(
                        ysb[pair][hp * 64:(hp + 1) * 64,
                                  qs * 512:(qs + 1) * 512],
                        yps[hp][0:64, :], rsb[:])
                for jj in (qs, qs + 4):
                    eng = nc.sync if jj % 2 == 0 else nc.gpsimd
                    eng.dma_start(
                        ybr2[pair][jj, :, :],
                        ysb[pair][:, qs * 512:(qs + 1) * 512])

            def fire_a2a(pair):
                if sim_mode:
                    nc.sync.dma_start(a2a_out[pair][0:P, :],
                                      ybounce[pair][0:P, :])
                else:
                    nc.gpsimd.collective_compute(
                        "AllToAll", mybir.AluOpType.bypass,
                        replica_groups=[list(range(NCORES))],
                        ins=[ybounce[pair][:].opt()],
                        outs=[a2a_out[pair][:].opt()])
                a2r = a2a_out[pair][:].rearrange("(j p) t -> p j t", p=P)
                (nc.sync if pair == 0 else nc.gpsimd).dma_start(
                    ylt[pair][:], a2r)

            if do_p1:
                # LN, qkv and attention interleaved per 512-token slice so
                # the LN chains (DVE/ACT) pipeline under attention of the
                # previous slice; each pair's A2A fires when its last
                # slice drains.
                for ts in range(NQ):
                    # v right after each tile's LN keeps PE fed while the
                    # next tile's LN chain (DVE/ACT) is still in flight
                    for i in range(4 * ts, 4 * ts + 4):
                        layernorm_transpose(xb_tile, 1, xlt, i, psmm)
                        v_tile(i)
                    qk_slice(ts, 0)
                    qk_slice(ts, 1)
                    if ts == 1:
                        issue_prefetch()
                    for pair in range(2):
                        attn_unit(ts, pair)
                        if ts == NQ - 1:
                            fire_a2a(pair)

        if not do_p2:
            with tc.tile_pool(name="dummy_out", bufs=1) as dpool:
                zt0 = dpool.tile([P, C], bf16, tag="zt0")
                nc.vector.memset(zt0[:], 0.0)
                for i in range(NT2):
                    nc.sync.dma_start(out[i * P:(i + 1) * P, :], zt0[:])
            nc.compile()
            return nc

        # ================= phase 2: residual + LN2 + MLP ================
        with tc.tile_pool(name="mlp_per", bufs=1) as mper:
            x2 = mper.tile([P, NT2, C], f32, tag="x2")
            x2lt = mper.tile([P, CS, TCHUNK], bf16, tag="x2lt")
            hT = mper.tile([P, NS, TCHUNK], bf16, tag="hT")

            with tc.tile_pool(name="psproj", bufs=4, space="PSUM") as psproj, \
                 tc.tile_pool(name="psmm2", bufs=2, space="PSUM") as psmm2:
                # proj pair-0 half: runs while A2A #1 is still in flight
                for i in range(NT2):
                    pss = [psproj.tile([P, 512], f32, tag="pj",
                                       name=f"pj0_{i}_{n}") for n in range(2)]
                    for j in range(CS):
                        for n in range(2):
                            nc.tensor.matmul(
                                pss[n][:],
                                lhsT=ylt[0][:, j, i * P:(i + 1) * P],
                                rhs=pw_s[:, 2 * j, n * 512:(n + 1) * 512],
                                start=(j == 0), stop=(j == CS - 1))
                    for n in range(2):
                        nc.vector.tensor_add(
                            x2[:, i, n * 512:(n + 1) * 512], pss[n][:],
                            ct[:, i, n * 512:(n + 1) * 512])
                # proj pair-1 half + LN2 interleaved
                for i in range(NT2):
                    pss = [psproj.tile([P, 512], f32, tag="pj",
                                       name=f"pj1_{i}_{n}") for n in range(2)]
                    for j in range(CS):
                        for n in range(2):
                            nc.tensor.matmul(
                                pss[n][:],
                                lhsT=ylt[1][:, j, i * P:(i + 1) * P],
                                rhs=pw_s[:, 2 * j + 1, n * 512:(n + 1) * 512],
                                start=(j == 0), stop=(j == CS - 1))
                    for n in range(2):
                        nc.vector.tensor_add(
                            x2[:, i, n * 512:(n + 1) * 512], pss[n][:],
                            x2[:, i, n * 512:(n + 1) * 512])
                    if i >= 1:
                        layernorm_transpose(lambda ii: x2[:, ii, :], 1,
                                            x2lt, i - 1, psmm2)
                layernorm_transpose(lambda ii: x2[:, ii, :], 1, x2lt,
                                    NT2 - 1, psmm2)

                # fc + gelu -> hT (feature-major); fw streamed in 16 chunks
                for g in range(16):
                    fwch = fw_chunks[g]
                    if g >= 8:
                        engs[g % 4].dma_start(fwch[:], fw[g])
                    for mq in range(2):
                        m = g * 2 + mq
                        ps = psmm2.tile([P, 512], f32, tag="mm")
                        for s in range(CS):
                            nc.tensor.matmul(
                                ps[:],
                                lhsT=fwch[:, s, mq * P:(mq + 1) * P],
                                rhs=x2lt[:, s, :],
                                start=(s == 0), stop=(s == CS - 1))
                        nc.scalar.activation(hT[:, m, :], ps[:], ACT.Gelu)

            # fc2: s-major, stationary hT tile reused across both halves,
            # 8 PSUM accumulator banks in a dedicated scope
            with tc.tile_pool(name="psfc2", bufs=8, space="PSUM") as psfc2:
                pss = [psfc2.tile([P, 512], f32, tag="fc2",
                                  name=f"fc2_{ti}_{n}")
                       for ti in range(NT2) for n in range(2)]
                outt = [work.tile([P, C], bf16, tag="ztw", name=f"ot_{t}")
                        for t in range(NT2)]
                for g in range(16):
                    f2ch = f2_chunks[g]
                    if g >= 2:
                        engs[g % 4].dma_start(f2ch[:], f2w[g])
                    for sq in range(2):
                        s = 2 * g + sq
                        for ti in range(NT2):
                            for n in range(2):
                                nc.tensor.matmul(
                                    pss[ti * 2 + n][:],
                                    lhsT=hT[:, s, ti * P:(ti + 1) * P],
                                    rhs=f2ch[:, sq, n * 512:(n + 1) * 512],
                                    start=(s == 0), stop=(s == NS - 1))
                            if s == NS - 1:
                                # evict each token tile as soon as its
                                # accumulation stops; out DMAs alternate
                                # queues so the final drain parallelizes
                                for n in range(2):
                                    nc.vector.tensor_add(
                                        outt[ti][:, n * 512:(n + 1) * 512],
                                        pss[ti * 2 + n][:],
                                        x2[:, ti, n * 512:(n + 1) * 512])
                                    eng = nc.sync if n == 0 else nc.gpsimd
                                    eng.dma_start(
                                        out[ti * P:(ti + 1) * P,
                                            n * 512:(n + 1) * 512],
                                        outt[ti][:, n * 512:(n + 1) * 512])

    nc.compile()
    return nc


def _prep_core_inputs(x, ln1_g, ln1_b, attn_w, attn_b, proj_w, proj_b,
                      ln2_g, ln2_b, fc_w, fc_b, fc2_w, fc2_b):
    """Host-side weight folding + per-core slicing. Returns in_maps list."""
    f = np.float32
    x = np.asarray(x, f)
    aw = np.asarray(ln1_g, f)[:, None] * np.asarray(attn_w, f)
    ab = np.asarray(attn_b, f) + np.asarray(ln1_b, f) @ np.asarray(attn_w, f)
    fwf = np.asarray(ln2_g, f)[:, None] * np.asarray(fc_w, f)
    fbf = np.asarray(fc_b, f) + np.asarray(ln2_b, f) @ np.asarray(fc_w, f)
    assert not np.any(ab) and not np.any(fbf), "nonzero qkv/fc bias unsupported"
    assert not np.any(np.asarray(proj_b, f)) and not np.any(
        np.asarray(fc2_b, f)), "nonzero proj/fc2 bias unsupported"

    qw = aw[:, :C] * f(1.0 / np.sqrt(HD))    # fold softmax scale into Wq
    kw = aw[:, C:2 * C]
    vw = aw[:, 2 * C:]
    pwf = np.asarray(proj_w, f)
    f2wf = np.asarray(fc2_w, f)

    def as_lhst(w):  # [K, N] -> [P, K//P, N]
        return np.ascontiguousarray(
            w.reshape(w.shape[0] // P, P, w.shape[1]).transpose(1, 0, 2)
        ).astype(BF)

    pw_pad = np.zeros((2, 2 * C, C), np.float32)
    for g in range(2):
        for j in range(NCORES):
            if j // TPG == g:
                r = j % TPG
                pw_pad[g, 256 * j:256 * (j + 1), :] = \
                    pwf[256 * r:256 * (r + 1), :]

    fw_l = as_lhst(fwf)            # [128, 8, 4096]
    fw_t = np.ascontiguousarray(
        np.stack([fw_l[:, :, g * 256:(g + 1) * 256] for g in range(16)]))
    f2_l = as_lhst(f2wf)           # [128, 32, 1024]
    f2w_t = np.ascontiguousarray(
        np.stack([f2_l[:, 2 * i:2 * i + 2, :] for i in range(16)]))

    # host-built constants: [P, P] identity + 4 causal diagonal masks
    ident = np.eye(P, dtype=np.float32)
    ki = np.arange(P)[:, None]
    qj = np.arange(512)[None, :]
    mask_list = [(ki - qj + 128 * j <= 0).astype(np.float32)
                 for j in range(4)]
    consts = np.concatenate([ident] + mask_list, axis=1).astype(BF)

    in_maps = []
    for core in range(NCORES):
        b, r = core // TPG, core % TPG
        cols = slice(256 * r, 256 * r + 256)
        in_maps.append({
            "xb": np.ascontiguousarray(x[b]).astype(BF),
            "xc": np.ascontiguousarray(x[b, TCHUNK * r:TCHUNK * (r + 1)]).astype(BF),
            "wq": as_lhst(qw[:, cols]),
            "wk": as_lhst(kw[:, cols]),
            "wv": as_lhst(vw[:, cols]),
            "pw": as_lhst(pw_pad[b]),
            "fw": fw_t,
            "f2w": f2w_t,
            "consts": consts,
        })
    return in_maps


_built = {}


def run(inputs, trace=False, debug=False, **spmd_kwargs):
    key = "rel"
    if key not in _built:
        _built[key] = build_nc(debug=debug)
    nc = _built[key]
    in_maps = _prep_core_inputs(**inputs)
    res = run_bass_kernel_spmd(nc, in_maps, list(range(NCORES)),
                               trace=trace, **spmd_kwargs)
    full = np.empty((B, T, C), np.float32)
    for core in range(NCORES):
        b, r = core // TPG, core % TPG
        full[b, TCHUNK * r:TCHUNK * (r + 1)] = res.results[core]["out"]
    return full, res


def kernel(**inputs):
    full, _ = run(inputs, trace=False, debug=False)
    return full


# revision 7
# speedup vs baseline: 1.3042x; 1.3042x over previous
"""Trainium2 Bass kernel for nn_Block_50130858279051 (dense transformer block).

Sharding: 8 cores = 2 batch groups x 4-way tensor parallel (as v1), with a
restructured schedule:
  - Attention runs PAIR-MAJOR: all 4 query slices for head-pair 0, fire its
    8-rank AllToAll, then pair 1 while the collective flies, then A2A #1.
  - Scores for the two heads of a pair are issued back-to-back into row
    groups 0/64 of the PE array (tile_position auto-derived), so they run
    concurrently; one [P,2,512] Exp covers both heads.
  - Diagonal k-tiles only process query columns >= 128*j (the causal-valid
    range), shrinking scores/exp/mask/AV on the diagonal by ~37%.
  - proj reuses each ylt lhsT across both 512-column halves (one LDWEIGHTS
    per two matmuls) and is split pair-0-first so its first half runs while
    A2A #1 is still in flight.
  - fc2 runs s-major with each hT stationary tile reused for both output
    halves, accumulating into 8 PSUM banks opened in their own pool scope.
  - All xb token blocks prefetch at kernel start (block 0 first).
"""
import sys

sys.path.insert(0, "/opt/trn_rl_repo")

import numpy as np
import ml_dtypes
from contextlib import ExitStack

import concourse.bacc as bacc
import concourse.mybir as mybir
import concourse.tile as tile
from concourse.bass_utils import run_bass_kernel_spmd

B, T, C, H, HD = 2, 2048, 1024, 16, 64
HID = 4 * C
P = 128
NCORES, TPG = 8, 4          # 2 groups x 4 cores
TCHUNK = T // TPG           # 512 tokens per core in the MLP phase
HPC = H // TPG              # 4 heads per core
CS = C // P                 # 8 channel subtiles
TT = T // P                 # 16 token tiles
NQ = T // 512               # 4 query slices of 512
NT2 = TCHUNK // P           # 4 token tiles in the chunk
NS = HID // P               # 32 hidden subtiles
f32, bf16 = mybir.dt.float32, mybir.dt.bfloat16
BF = ml_dtypes.bfloat16
ACT = mybir.ActivationFunctionType


def build_nc(debug=False, sim_mode=False, do_p1=True, do_p2=True):
    nc = bacc.Bacc("TRN2", target_bir_lowering=False, debug=False,
                   num_devices=NCORES, num_swdge_queues=4)
    xb = nc.declare_dram_parameter("xb", [T, C], bf16, isOutput=False)
    xc = nc.declare_dram_parameter("xc", [TCHUNK, C], bf16, isOutput=False)
    wq = nc.declare_dram_parameter("wq", [P, CS, 2 * P], bf16, isOutput=False)
    wk = nc.declare_dram_parameter("wk", [P, CS, 2 * P], bf16, isOutput=False)
    wv = nc.declare_dram_parameter("wv", [P, CS, 2 * P], bf16, isOutput=False)
    pw = nc.declare_dram_parameter("pw", [P, 2 * CS, C], bf16, isOutput=False)
    fw = nc.declare_dram_parameter("fw", [16, P, CS, 256], bf16,
                                   isOutput=False)
    f2w = nc.declare_dram_parameter("f2w", [16, P, 2, C], bf16,
                                    isOutput=False)
    consts = nc.declare_dram_parameter("consts", [P, P + 4 * 512], bf16,
                                       isOutput=False)
    out = nc.declare_dram_parameter("out", [TCHUNK, C], bf16, isOutput=True)

    with tile.TileContext(nc) as tc, ExitStack() as ctx:
        per = ctx.enter_context(tc.tile_pool(name="persist", bufs=1))
        work = ctx.enter_context(tc.tile_pool(name="work", bufs=3))
        small = ctx.enter_context(tc.tile_pool(name="small", bufs=3))
        dram = ctx.enter_context(tc.tile_pool(name="dram", bufs=1,
                                              space="DRAM"))

        # ---- constants: ident + causal diagonal masks --------------------
        # ident rides first (the first LN tile needs it within ~3us);
        # masks aren't needed until attention starts.
        cst = per.tile([P, P + 4 * 512], bf16, tag="cst")
        nc.gpsimd.dma_start(cst[:, 0:P], consts[:, 0:P])
        ident = cst[:, 0:P]
        masks = cst[:, P:].rearrange("p (j t) -> p j t", j=4)
        eps_t = per.tile([P, 1], f32, tag="eps")
        nc.vector.memset(eps_t[:], 1e-5)

        fws = ctx.enter_context(tc.tile_pool(name="fwstream", bufs=2))
        f2s = ctx.enter_context(tc.tile_pool(name="f2stream", bufs=2))
        ypool = ctx.enter_context(tc.tile_pool(name="ypool", bufs=1))
        engs = [nc.gpsimd, nc.gpsimd, nc.sync, nc.sync]
        fw_chunks = [fws.tile([P, CS, 256], bf16, tag="fwch", name=f"fwch{g}")
                     for g in range(16)]
        f2_chunks = [f2s.tile([P, 2, C], bf16, tag="f2ch", name=f"f2ch{i}")
                     for i in range(16)]

        ylt = [ypool.tile([P, CS, TCHUNK], bf16, tag=f"ylt{pr}",
                          name=f"ylt{pr}") for pr in range(2)]
        ct = ypool.tile([P, NT2, C], bf16, tag="xct")
        pw_s = ypool.tile([P, 2 * CS, C], bf16, tag="pw")

        def issue_prefetch():
            for g in range(8):
                engs[g % 4].dma_start(fw_chunks[g][:], fw[g])
            for idx in range(2):
                engs[idx % 4].dma_start(f2_chunks[idx][:], f2w[idx])
            nc.sync.dma_start(
                ct[:], xc[:].rearrange("(i p) c -> p i c", p=P))
            pwr_d = pw[:].rearrange("p (j two) c -> p two j c", two=2)
            pwr_s = pw_s[:].rearrange("p (j two) c -> p two j c", two=2)
            nc.gpsimd.dma_start(pwr_s[:, 0], pwr_d[:, 0])
            nc.sync.dma_start(pwr_s[:, 1], pwr_d[:, 1])

        ybounce = [dram.tile([NCORES * P, TCHUNK], bf16, name=f"ybounce{i}",
                             tag=f"ybounce{i}") for i in range(2)]
        a2a_out = [dram.tile([NCORES * P, TCHUNK], bf16, name=f"a2a_out{i}",
                             tag=f"a2a_out{i}") for i in range(2)]

        def layernorm_transpose(get_tile, n_tiles, dst, i0, pp):
            """Token-major [P, C] tiles -> feature-major bf16 dst."""
            for i in range(i0, i0 + n_tiles):
                xt = get_tile(i)
                stats = small.tile([P, 2, 6], f32, tag="s_bn")
                mv = small.tile([P, 2], f32, tag="s_mv")
                std = small.tile([P, 1], f32, tag="s_std")
                rstd = small.tile([P, 1], f32, tag="s_rstd")
                negmu = small.tile([P, 1], f32, tag="s_negmu")
                cen = work.tile([P, C], bf16, tag="cen")
                for g in range(2):
                    nc.vector.bn_stats(stats[:, g, :],
                                       xt[:, g * 512:(g + 1) * 512])
                nc.vector.bn_aggr(mv[:],
                                  stats[:].rearrange("p g s -> p (g s)"))
                nc.scalar.activation(std[:], mv[:, 1:2], ACT.Sqrt,
                                     bias=eps_t[:])
                nc.vector.reciprocal(rstd[:], std[:])
                nc.vector.tensor_scalar_mul(negmu[:], mv[:, 0:1], -1.0)
                # centered copy alternates ACT/DVE to balance engine load
                if i % 2 == 0:
                    nc.scalar.activation(cen[:], xt[:], ACT.Identity,
                                         bias=negmu[:])
                else:
                    nc.vector.tensor_scalar_add(cen[:], xt[:], negmu[:])
                dmat = work.tile([P, P], bf16, tag="dmat")
                nc.vector.tensor_scalar_mul(dmat[:], ident, rstd[:])
                for half in range(2):
                    ps = pp.tile([P, 512], f32, tag="mm")
                    for cq in range(4):
                        c = half * 4 + cq
                        nc.tensor.matmul(
                            ps[:, cq * P:(cq + 1) * P],
                            lhsT=cen[:, c * P:(c + 1) * P],
                            rhs=dmat[:], start=True, stop=True)
                    eng = nc.vector if half == 0 else nc.scalar
                    if eng is nc.vector:
                        eng.tensor_copy(
                            out=dst[:, half * 4:(half + 1) * 4,
                                    i * P:(i + 1) * P],
                            in_=ps[:].rearrange("p (c t) -> p c t", c=4))
                    else:
                        eng.copy(
                            dst[:, half * 4:(half + 1) * 4,
                                i * P:(i + 1) * P],
                            ps[:].rearrange("p (c t) -> p c t", c=4))

        # ================= phase 1: LN1, QKV, attention =================
        with tc.tile_pool(name="xlt_pool", bufs=1) as xlt_pool, \
             tc.tile_pool(name="at_pool", bufs=3) as at_pool, \
             tc.tile_pool(name="xb_pool", bufs=3) as xb_pool, \
             tc.tile_pool(name="psy", bufs=2, space="PSUM") as psy, \
             tc.tile_pool(name="psatt", bufs=2, space="PSUM") as psatt, \
             tc.tile_pool(name="psmm", bufs=2, space="PSUM") as psmm:
            xlt = xlt_pool.tile([P, CS, T], bf16, tag="xlt")
            vA = xlt_pool.tile([P, TT, HPC, 65], bf16, tag="vA")
            nc.vector.memset(vA[:, :, :, 64:65], 1.0)
            qT = [xlt_pool.tile([P, T], bf16, tag=f"qT{p}", name=f"qT{p}")
                  for p in range(2)]
            kT = [xlt_pool.tile([P, T], bf16, tag=f"kT{p}", name=f"kT{p}")
                  for p in range(2)]
            ysb = [xlt_pool.tile([P, T], bf16, tag=f"ysb{p}",
                                 name=f"ysb{p}") for p in range(2)]
            wq_s = xlt_pool.tile([P, CS, 2 * P], bf16, tag="wq")
            wk_s = xlt_pool.tile([P, CS, 2 * P], bf16, tag="wk")
            wv_s = xlt_pool.tile([P, CS, 2 * P], bf16, tag="wv")

            # startup DMA: first xb block, qkv weights, then the rest of xb.
            # blocks rotate through 3 buffers; block 3's DMA waits for
            # block 0's consumers automatically.
            # qkv weights ride the gpsimd queue (behind consts only, wv
            # first since v-tiles consume it first); all xb blocks ride
            # the sync queue so they never delay the weights.
            xb_blocks = [xb_pool.tile([P, 4, C], bf16, tag="xbblk",
                                      name=f"xbblk{b}") for b in range(4)]
            xbr = xb[:].rearrange("(blk i p) c -> blk p i c", blk=4, p=P)
            nc.sync.dma_start(xb_blocks[0][:, 0:1, :], xbr[0][:, 0:1, :])
            # PE warm-up: dependency-free matmuls on the identity block fill
            # the otherwise-idle window while the first LN chain's DVE/ACT
            # latency drains, and push the PE clock gate to full rate before
            # the first real matmuls issue.
            wups = psmm.tile([64, P], f32, tag="mm", name="warmup_ps")
            for wi in range(24):
                nc.tensor.matmul(wups[:], lhsT=ident[0:64, 0:64],
                                 rhs=ident[0:64, :],
                                 start=(wi == 0), stop=(wi == 23))
            nc.gpsimd.dma_start(wv_s[:], wv[:])
            nc.gpsimd.dma_start(wq_s[:], wq[:])
            nc.gpsimd.dma_start(wk_s[:], wk[:])
            nc.gpsimd.dma_start(cst[:, P:], consts[:, P:])
            nc.sync.dma_start(xb_blocks[0][:, 1:4, :], xbr[0][:, 1:4, :])
            for blk in range(1, 4):
                nc.sync.dma_start(xb_blocks[blk][:], xbr[blk])

            def xb_tile(i):
                return xb_blocks[i // 4][:, i % 4, :]

            def qk_slice(ts, pair):
                for dst_t, wsb in ((qT[pair], wq_s), (kT[pair], wk_s)):
                    ps = psmm.tile([P, 512], f32, tag="mm")
                    for s in range(CS):
                        nc.tensor.matmul(
                            ps[:],
                            lhsT=wsb[:, s, pair * P:(pair + 1) * P],
                            rhs=xlt[:, s, ts * 512:(ts + 1) * 512],
                            start=(s == 0), stop=(s == CS - 1))
                    nc.vector.tensor_copy(
                        out=dst_t[:, ts * 512:(ts + 1) * 512],
                        in_=ps[:])

            def v_tile(ti):
                ps = psmm.tile([P, 512], f32, tag="mm")
                for s in range(CS):
                    nc.tensor.matmul(
                        ps[:, :2 * P],
                        lhsT=xlt[:, s, ti * P:(ti + 1) * P],
                        rhs=wv_s[:, s, :],
                        start=(s == 0), stop=(s == CS - 1))
                nc.vector.tensor_copy(
                    out=vA[:, ti, :, 0:64],
                    in_=ps[:, :2 * P].rearrange("p (h d) -> p h d",
                                                h=HPC))

            ybr2 = [yb[:].rearrange("(j p) t -> j p t", j=NCORES)
                    for yb in ybounce]

            def attn_unit(qs, pair):
                """Scores (row-packed pair), exp, AV (2 k-tiles behind so
                the packed scores + prior AV cover the exp latency),
                normalize, ship."""
                nkt = 4 * qs + 4
                yps = [psy.tile([65, 512], f32, tag="yps",
                                name=f"yps_{qs}_{pair}_{hp}")
                       for hp in range(2)]

                def av(at, kt, qlo):
                    for hp in range(2):
                        nc.tensor.matmul(
                            yps[hp][:, qlo:],
                            lhsT=vA[:, kt, 2 * pair + hp, :],
                            rhs=at[:, hp, qlo:],
                            start=(kt == 0), stop=(kt == nkt - 1))

                pipe = []
                for kt in range(nkt):
                    j = kt - 4 * qs
                    qlo = max(0, 128 * j)
                    sps = psatt.tile([P, 2, 512], f32, tag="satt")
                    for hp in range(2):
                        nc.tensor.matmul(
                            sps[:, hp, qlo:],
                            lhsT=kT[pair][hp * 64:(hp + 1) * 64,
                                          kt * P:(kt + 1) * P],
                            rhs=qT[pair][hp * 64:(hp + 1) * 64,
                                         qs * 512 + qlo:(qs + 1) * 512],
                            start=True, stop=True)
                    if len(pipe) >= 2:
                        av(*pipe.pop(0))
                    at = at_pool.tile([P, 2, 512], bf16, tag="at")
                    nc.scalar.activation(at[:, :, qlo:], sps[:, :, qlo:],
                                         ACT.Exp)
                    if j >= 0:
                        for hp in range(2):
                            nc.vector.tensor_mul(at[:, hp, qlo:],
                                                 at[:, hp, qlo:],
                                                 masks[:, j, qlo:])
                    pipe.append((at, kt, qlo))
                for p in pipe:
                    av(*p)
                for hp in range(2):
                    lrec = small.tile([1, 512], f32, tag="lrec")
                    nc.vector.reciprocal(lrec[:], yps[hp][64:65, :])
                    rsb = work.tile([64, 512], f32, tag="rsb")
                    nc.gpsimd.partition_broadcast(rsb[:], lrec[:])
                    nc.vector.tensor_mul(
                        ysb[pair][hp * 64:(hp + 1) * 64,
                                  qs * 512:(qs + 1) * 512],
                        yps[hp][0:64, :], rsb[:])
                for jj in (qs, qs + 4):
                    eng = nc.sync if jj % 2 == 0 else nc.gpsimd
                    eng.dma_start(
                        ybr2[pair][jj, :, :],
                        ysb[pair][:, qs * 512:(qs + 1) * 512])

            def fire_a2a(pair):
                if sim_mode:
                    nc.sync.dma_start(a2a_out[pair][0:P, :],
                                      ybounce[pair][0:P, :])
                else:
                    nc.gpsimd.collective_compute(
                        "AllToAll", mybir.AluOpType.bypass,
                        replica_groups=[list(range(NCORES))],
                        ins=[ybounce[pair][:].opt()],
                        outs=[a2a_out[pair][:].opt()])
                # recv split across both queues so ylt lands sooner
                a2r = a2a_out[pair][:].rearrange("(j p) t -> p j t", p=P)
                nc.sync.dma_start(ylt[pair][:, 0:CS // 2], a2r[:, 0:CS // 2])
                nc.gpsimd.dma_start(ylt[pair][:, CS // 2:], a2r[:, CS // 2:])

            if do_p1:
                # LN, qkv and attention interleaved per 512-token slice so
                # the LN chains (DVE/ACT) pipeline under attention of the
                # previous slice; each pair's A2A fires when its last
                # slice drains.
                for ts in range(NQ):
                    # v right after each tile's LN keeps PE fed while the
                    # next tile's LN chain (DVE/ACT) is still in flight
                    for i in range(4 * ts, 4 * ts + 4):
                        layernorm_transpose(xb_tile, 1, xlt, i, psmm)
                        v_tile(i)
                    qk_slice(ts, 0)
                    qk_slice(ts, 1)
                    if ts == 1:
                        issue_prefetch()
                    for pair in range(2):
                        attn_unit(ts, pair)
                        if ts == NQ - 1:
                            fire_a2a(pair)

        if not do_p2:
            with tc.tile_pool(name="dummy_out", bufs=1) as dpool:
                zt0 = dpool.tile([P, C], bf16, tag="zt0")
                nc.vector.memset(zt0[:], 0.0)
                for i in range(NT2):
                    nc.sync.dma_start(out[i * P:(i + 1) * P, :], zt0[:])
            nc.compile()
            return nc

        # ================= phase 2: residual + LN2 + MLP ================
        with tc.tile_pool(name="mlp_per", bufs=1) as mper:
            x2 = mper.tile([P, NT2, C], f32, tag="x2")
            x2lt = mper.tile([P, CS, TCHUNK], bf16, tag="x2lt")
            hT = mper.tile([P, NS, TCHUNK], bf16, tag="hT")

            with tc.tile_pool(name="psproj", bufs=4, space="PSUM") as psproj, \
                 tc.tile_pool(name="psmm2", bufs=2, space="PSUM") as psmm2:
                # proj pair-0 half: runs while A2A #1 is still in flight
                for i in range(NT2):
                    pss = [psproj.tile([P, 512], f32, tag="pj",
                                       name=f"pj0_{i}_{n}") for n in range(2)]
                    for j in range(CS):
                        for n in range(2):
                            nc.tensor.matmul(
                                pss[n][:],
                                lhsT=ylt[0][:, j, i * P:(i + 1) * P],
                                rhs=pw_s[:, 2 * j, n * 512:(n + 1) * 512],
                                start=(j == 0), stop=(j == CS - 1))
                    for n in range(2):
                        nc.vector.tensor_add(
                            x2[:, i, n * 512:(n + 1) * 512], pss[n][:],
                            ct[:, i, n * 512:(n + 1) * 512])
                # proj pair-1 half + LN2 interleaved
                for i in range(NT2):
                    pss = [psproj.tile([P, 512], f32, tag="pj",
                                       name=f"pj1_{i}_{n}") for n in range(2)]
                    for j in range(CS):
                        for n in range(2):
                            nc.tensor.matmul(
                                pss[n][:],
                                lhsT=ylt[1][:, j, i * P:(i + 1) * P],
                                rhs=pw_s[:, 2 * j + 1, n * 512:(n + 1) * 512],
                                start=(j == 0), stop=(j == CS - 1))
                    for n in range(2):
                        nc.vector.tensor_add(
                            x2[:, i, n * 512:(n + 1) * 512], pss[n][:],
                            x2[:, i, n * 512:(n + 1) * 512])
                    if i >= 1:
                        layernorm_transpose(lambda ii: x2[:, ii, :], 1,
                                            x2lt, i - 1, psmm2)
                layernorm_transpose(lambda ii: x2[:, ii, :], 1, x2lt,
                                    NT2 - 1, psmm2)

                # fc + gelu -> hT (feature-major); fw streamed in 16 chunks
                for g in range(16):
                    fwch = fw_chunks[g]
                    if g >= 8:
                        engs[g % 4].dma_start(fwch[:], fw[g])
                    for mq in range(2):
                        m = g * 2 + mq
                        ps = psmm2.tile([P, 512], f32, tag="mm")
                        for s in range(CS):
                            nc.tensor.matmul(
                                ps[:],
                                lhsT=fwch[:, s, mq * P:(mq + 1) * P],
                                rhs=x2lt[:, s, :],
                                start=(s == 0), stop=(s == CS - 1))
                        nc.scalar.activation(hT[:, m, :], ps[:], ACT.Gelu)

            # fc2: s-major, stationary hT tile reused across both halves,
            # 8 PSUM accumulator banks in a dedicated scope
            with tc.tile_pool(name="psfc2", bufs=8, space="PSUM") as psfc2:
                pss = [psfc2.tile([P, 512], f32, tag="fc2",
                                  name=f"fc2_{ti}_{n}")
                       for ti in range(NT2) for n in range(2)]
                outt = [work.tile([P, C], bf16, tag="ztw", name=f"ot_{t}")
                        for t in range(NT2)]
                for g in range(15):
                    f2ch = f2_chunks[g]
                    if g >= 2:
                        engs[g % 4].dma_start(f2ch[:], f2w[g])
                    for sq in range(2):
                        s = 2 * g + sq
                        for ti in range(NT2):
                            for n in range(2):
                                nc.tensor.matmul(
                                    pss[ti * 2 + n][:],
                                    lhsT=hT[:, s, ti * P:(ti + 1) * P],
                                    rhs=f2ch[:, sq, n * 512:(n + 1) * 512],
                                    start=(s == 0), stop=False)
                # last chunk runs token-tile-major so each tile's eviction
                # and output DMA overlap the remaining tiles' matmuls
                # instead of all draining at the very end
                f2ch = f2_chunks[15]
                engs[15 % 4].dma_start(f2ch[:], f2w[15])
                for ti in range(NT2):
                    for sq in range(2):
                        s = 30 + sq
                        for n in range(2):
                            nc.tensor.matmul(
                                pss[ti * 2 + n][:],
                                lhsT=hT[:, s, ti * P:(ti + 1) * P],
                                rhs=f2ch[:, sq, n * 512:(n + 1) * 512],
                                start=False, stop=(s == NS - 1))
                    for n in range(2):
                        nc.vector.tensor_add(
                            outt[ti][:, n * 512:(n + 1) * 512],
                            pss[ti * 2 + n][:],
                            x2[:, ti, n * 512:(n + 1) * 512])
                        eng = nc.sync if n == 0 else nc.gpsimd
                        eng.dma_start(
                            out[ti * P:(ti + 1) * P,
                                n * 512:(n + 1) * 512],
                            outt[ti][:, n * 512:(n + 1) * 512])

    nc.compile()
    return nc


def _prep_core_inputs(x, ln1_g, ln1_b, attn_w, attn_b, proj_w, proj_b,
                      ln2_g, ln2_b, fc_w, fc_b, fc2_w, fc2_b):
    """Host-side weight folding + per-core slicing. Returns in_maps list."""
    f = np.float32
    x = np.asarray(x, f)
    aw = np.asarray(ln1_g, f)[:, None] * np.asarray(attn_w, f)
    ab = np.asarray(attn_b, f) + np.asarray(ln1_b, f) @ np.asarray(attn_w, f)
    fwf = np.asarray(ln2_g, f)[:, None] * np.asarray(fc_w, f)
    fbf = np.asarray(fc_b, f) + np.asarray(ln2_b, f) @ np.asarray(fc_w, f)
    assert not np.any(ab) and not np.any(fbf), "nonzero qkv/fc bias unsupported"
    assert not np.any(np.asarray(proj_b, f)) and not np.any(
        np.asarray(fc2_b, f)), "nonzero proj/fc2 bias unsupported"

    qw = aw[:, :C] * f(1.0 / np.sqrt(HD))    # fold softmax scale into Wq
    kw = aw[:, C:2 * C]
    vw = aw[:, 2 * C:]
    pwf = np.asarray(proj_w, f)
    f2wf = np.asarray(fc2_w, f)

    def as_lhst(w):  # [K, N] -> [P, K//P, N]
        return np.ascontiguousarray(
            w.reshape(w.shape[0] // P, P, w.shape[1]).transpose(1, 0, 2)
        ).astype(BF)

    pw_pad = np.zeros((2, 2 * C, C), np.float32)
    for g in range(2):
        for j in range(NCORES):
            if j // TPG == g:
                r = j % TPG
                pw_pad[g, 256 * j:256 * (j + 1), :] = \
                    pwf[256 * r:256 * (r + 1), :]

    fw_l = as_lhst(fwf)            # [128, 8, 4096]
    fw_t = np.ascontiguousarray(
        np.stack([fw_l[:, :, g * 256:(g + 1) * 256] for g in range(16)]))
    f2_l = as_lhst(f2wf)           # [128, 32, 1024]
    f2w_t = np.ascontiguousarray(
        np.stack([f2_l[:, 2 * i:2 * i + 2, :] for i in range(16)]))

    # host-built constants: [P, P] identity + 4 causal diagonal masks
    ident = np.eye(P, dtype=np.float32)
    ki = np.arange(P)[:, None]
    qj = np.arange(512)[None, :]
    mask_list = [(ki - qj + 128 * j <= 0).astype(np.float32)
                 for j in range(4)]
    consts = np.concatenate([ident] + mask_list, axis=1).astype(BF)

    in_maps = []
    for core in range(NCORES):
        b, r = core // TPG, core % TPG
        cols = slice(256 * r, 256 * r + 256)
        in_maps.append({
            "xb": np.ascontiguousarray(x[b]).astype(BF),
            "xc": np.ascontiguousarray(x[b, TCHUNK * r:TCHUNK * (r + 1)]).astype(BF),
            "wq": as_lhst(qw[:, cols]),
            "wk": as_lhst(kw[:, cols]),
            "wv": as_lhst(vw[:, cols]),
            "pw": as_lhst(pw_pad[b]),
            "fw": fw_t,
            "f2w": f2w_t,
            "consts": consts,
        })
    return in_maps


_built = {}


def run(inputs, trace=False, debug=False, **spmd_kwargs):
    key = "rel"
    if key not in _built:
        _built[key] = build_nc(debug=debug)
    nc = _built[key]
    in_maps = _prep_core_inputs(**inputs)
    res = run_bass_kernel_spmd(nc, in_maps, list(range(NCORES)),
                               trace=trace, **spmd_kwargs)
    full = np.empty((B, T, C), np.float32)
    for core in range(NCORES):
        b, r = core // TPG, core % TPG
        full[b, TCHUNK * r:TCHUNK * (r + 1)] = res.results[core]["out"]
    return full, res


def kernel(**inputs):
    full, _ = run(inputs, trace=False, debug=False)
    return full
